# revision 58
# baseline (speedup 1.0000x reference)
"""Trainium2 Bass kernel for nn_BinaryConv2d (B=16, C=64, H=W=256, 3x3, pad 1).

Forward semantics (STE forward values):
  act = sign(x * rd_k + rd_b)                  in {-1, 0, +1}
  bw  = scaling[co] * sign(conv_w)             scaling = mean |conv_w| per out-ch
  y   = conv2d(act, bw, pad=1)
  y   = prelu(y + pr_bias0) + pr_bias1 + x     prelu slope per channel

Strategy: data-parallel over batch, 2 images per core (8 cores).  The two
images' 64 channels are stacked on the 128 SBUF partitions.  x is shipped as
bf16 (residual-precision is ample for the 2e-2 gate) and y is returned as
bf16, halving HBM traffic.  Activations are binarized to fp8 +-1 on the
Scalar engine; the 3x3 conv is 5 PSUM matmuls per output row with
block-diagonal +-1 fp8 weights (exact integer arithmetic in fp32 PSUM):
3 DoubleRow matmuls pair kh=0/1 per kw, a 4th DoubleRow pairs the kh=2
kw=0/1 taps via an overlapping stride-1 rhs AP, and kh=2,kw=2 is a plain
matmul.  Matmuls are emitted weight-major within each 4-row PSUM group so
one LDWEIGHTS covers 4 row-matmuls.  Post-ops per 4-row group: ACT drains
PSUM (v = s*ps + b0, bf16), DVE does the prelu as one stt max(v, slope*v),
and the residual add alternates DVE/Pool.  Measured-cost notes: GPSIMD
cannot read PSUM; Lrelu is broken on TRN2 hw; DVE stt ~1.6ns/elem, Pool
tensor ops ~2.8ns/elem, ACT ~0.9ns/elem incl. overhead.
"""

import sys

if "/opt/trn_rl_repo" not in sys.path:
    sys.path.insert(0, "/opt/trn_rl_repo")

from contextlib import ExitStack

import ml_dtypes
import numpy as np

import concourse.bacc as bacc
import concourse.bass as bass
import concourse.tile as tile
from concourse import mybir
from concourse.ap import AP
from concourse.bass_utils import run_bass_kernel_spmd

B, C, H, W = 16, 64, 256, 256
NCORES = 8
P = 128                      # partitions = 2 images x 64 channels

F32 = mybir.dt.float32
BF16 = mybir.dt.bfloat16
FP8 = mybir.dt.float8e4
AF = mybir.ActivationFunctionType
ALU = mybir.AluOpType
DR = mybir.MatmulPerfMode.DoubleRow

APITCH = 272                 # act row pitch (bytes %16 for DoubleRow AP steps)

# Param table columns (per-partition f32 scalars)
PK, PB, PS, PB0, PCM, PSL, PSAM, PZ = range(8)

# kh=2 row handled as DoubleRow over (kw=0, kw=1) with an overlapping
# stride-1 rhs AP + one plain kw=2 matmul (10W streamed columns per 2-row
# tile).  False falls back to one plain 2-row matmul per kw (12W).
KH2_DR = True
# Fast path (requires pr_bias0 == 0 and 1e-3 <= slope <= 1, true for this
# model's init): prelu(v) = slope*v + (1-slope)*relu(v), so ACT drains
# r = Relu(ps * s*(1-slope)) once, the residual x is folded into PSUM by a
# diag matmul (ps += diag(1/(slope*s)) @ x), and a single DVE stt emits
# y = slope*s*ps + r directly.  kernel() falls back otherwise.
FAST_POST = True
# x-fold engine pattern: groups with g % RES_DEN < RES_NUM put the
# residual through the PE diag matmul, the rest through a DVE stt pair
RES_NUM, RES_DEN = 3, 16

# strip heights (sum == H, all % 4 == 0): small first strip for fast
# pipeline fill, small last strip for a short drain/store tail
STRIP_HS = [32, 36, 36, 36, 36, 36, 36, 8]


def _sign_chunks(nrows, first):
    """Progressive chunk sizes covering nrows of sign activation.  The
    first chunk covers exactly the act rows the strip's first conv group
    consumes, so that group never waits on a later chunk."""
    chunks = [first]
    left = nrows - first
    while left > 0:
        c = min(16, left)
        chunks.append(c)
        left -= c
    return tuple(chunks)


def _emit(tc, nc, x_d, w_d, p_d, y_d, wx_d):
    x3 = x_d.rearrange("p (h w) -> p h w", w=W)
    y3 = y_d.rearrange("p (h w) -> p h w", w=W)

    with ExitStack() as ctx:
        consts = ctx.enter_context(tc.tile_pool(name="consts", bufs=1))
        xpool = ctx.enter_context(tc.tile_pool(name="xpool", bufs=4))
        apool = ctx.enter_context(tc.tile_pool(name="apool", bufs=2))
        ypool = ctx.enter_context(tc.tile_pool(name="ypool", bufs=2))
        vpool = ctx.enter_context(tc.tile_pool(name="vpool", bufs=8))
        mpool = ctx.enter_context(tc.tile_pool(name="mpool", bufs=4))
        upool = ctx.enter_context(tc.tile_pool(name="upool", bufs=8))
        pspool = ctx.enter_context(tc.tile_pool(name="pspool", bufs=4,
                                                space="PSUM"))

        # params first on the load ring (sign needs them); weights on the
        # store ring, which is idle at kernel start
        pt = consts.tile([P, 8], F32)
        nc.sync.dma_start(out=pt, in_=p_d)
        # [kw, delta(kh 0/1), m] DoubleRow weights; kw=0 ships alone so the
        # very first matmul isn't gated on the full weight load
        wdr = consts.tile([P, 3, 2, 128], FP8)
        nc.scalar.dma_start(out=wdr[:, 0], in_=w_d[:, :256].rearrange(
            "p (d m) -> p d m", d=2))
        nc.scalar.dma_start(out=wdr[:, 1:], in_=w_d[:, 256:768].rearrange(
            "p (k d m) -> p k d m", k=2, d=2))
        # kh=2: [delta(kw 0/1), m] DoubleRow + kw=2 plain
        wk2 = consts.tile([P, 2, 128], FP8)
        nc.scalar.dma_start(out=wk2, in_=w_d[:, 768:1024].rearrange(
            "p (d m) -> p d m", d=2))
        wn2 = consts.tile([P, 128], FP8)
        nc.scalar.dma_start(out=wn2, in_=w_d[:, 1024:])
        # diag(1/(slope*s)) for folding the residual into PSUM via the PE
        wx = consts.tile([P, 128], BF16)
        nc.scalar.dma_start(out=wx, in_=wx_d)

        H0S = [sum(STRIP_HS[:i]) for i in range(len(STRIP_HS))]
        NST = len(STRIP_HS)
        HSMAX = max(STRIP_HS)

        def strip_rows(s):
            h0 = H0S[s]
            row_lo = max(h0 - 1, 0)
            row_hi = min(h0 + STRIP_HS[s] + 1, H)
            return h0, row_lo, row_hi, row_lo - (h0 - 1)

        def load_strip(s):
            """DMA the x strip (rows h0-1 .. h0+hs; tile row a <-> global
            h0-1+a) and memset the act padding."""
            h0, row_lo, row_hi, r0 = strip_rows(s)
            nr = row_hi - row_lo
            xs = xpool.tile([P, HSMAX + 2, W], BF16, name="xs")
            if s == 0:
                # first transfer covers exactly the rows conv group 0 needs
                bounds = [0, 6]
                while bounds[-1] < nr:
                    bounds.append(min(bounds[-1] + 9, nr))
            else:
                # a small first transfer lets the strip's first sign chunk
                # (6 rows) start before the bulk of the strip lands
                bounds = [row_lo, row_lo + 8, row_lo + (nr + 8) // 2,
                          row_lo + nr]
            for idx, (a, b) in enumerate(zip(bounds, bounds[1:])):
                if b > a:
                    # first startup chunk issues from the Pool ring so it
                    # doesn't queue behind the param-table load
                    eng = nc.gpsimd if (s == 0 and idx == 0) else nc.sync
                    eng.dma_start(out=xs[:, a - (h0 - 1):b - (h0 - 1), :],
                                  in_=x3[:, a:b, :])
            act = apool.tile([P, HSMAX + 2, APITCH], FP8, name="act")
            nrows = STRIP_HS[s] + 2
            nc.gpsimd.memset(act[:, :nrows, 0:1], 0.0)
            nc.gpsimd.memset(act[:, :nrows, W + 1:W + 2], 0.0)
            if s == 0:
                nc.gpsimd.memset(act[:, 0:1, :], 0.0)
            if s == NST - 1:
                nc.gpsimd.memset(act[:, nrows - 1:nrows, :], 0.0)
            return xs, act

        def sign_chunk(s, xs, act, c0, sz):
            """Binarize rows [c0, c0+sz) of the strip's x tile into the
            zero-padded act tile."""
            _, row_lo, row_hi, r0 = strip_rows(s)
            lo = r0 + c0
            hi = min(lo + sz, r0 + (row_hi - row_lo))
            if hi > lo:
                nc.scalar.activation(
                    act[:, lo:hi, 1:W + 1], xs[:, lo:hi, :], AF.Sign,
                    bias=pt[:, PB:PB + 1], scale=pt[:, PK:PK + 1],
                )

        def kh2_overlap_rhs(act, row):
            """[P, 2, W] rhs with both free strides 1: element (d, m) reads
            padded act col d+m of `row`, pairing the kh=2 kw=0/1 taps."""
            base = act[:, row, 0:W]
            return AP(base.tensor, base.offset,
                      [list(base.ap[0]), [1, 2], [1, W]])

        def conv_group(act, ps4, r0):
            """All matmuls for 4 output rows (act tap rows r0..r0+5) in
            weight-major order: consecutive matmuls share a weight set.
            Banks (rows 0-1 / 2-3) start on their first matmul and stop on
            their last.  (Row-pair DR batching via 4-dim rhs APs doesn't
            lower: the 272-element act pitch prevents the free dims from
            flattening to the [P, 2, N] DoubleRow shape.)"""
            for kw in range(3):
                for i in range(4):
                    nc.tensor.matmul(
                        ps4[:, i, :], lhsT=wdr[:, kw, :, :],
                        rhs=act[:, r0 + i:r0 + i + 2, kw:kw + W],
                        start=(kw == 0 and i % 2 == 0),
                        stop=False, perf_mode=DR,
                    )
            if KH2_DR:
                for i in range(4):
                    nc.tensor.matmul(
                        ps4[:, i, :], lhsT=wk2,
                        rhs=kh2_overlap_rhs(act, r0 + i + 2),
                        start=False, stop=False, perf_mode=DR,
                    )
                for half in range(2):
                    nc.tensor.matmul(
                        ps4[:, 2 * half:2 * half + 2, :], lhsT=wn2,
                        rhs=act[:, r0 + 2 * half + 2:r0 + 2 * half + 4,
                                2:2 + W],
                        start=False, stop=True,
                    )
            else:
                for kw in range(3):
                    wsel = wk2[:, 0, :] if kw == 0 else (
                        wk2[:, 1, :] if kw == 1 else wn2)
                    for half in range(2):
                        nc.tensor.matmul(
                            ps4[:, 2 * half:2 * half + 2, :], lhsT=wsel,
                            rhs=act[:, r0 + 2 * half + 2:r0 + 2 * half + 4,
                                    kw:kw + W],
                            start=False, stop=(kw == 2),
                        )

        def post_general(g, xs, ys, ps4, r0):
            """General path: ACT Identity drain, DVE min/mult prelu, DVE
            residual (bf16 intermediates: DVE lane bandwidth is ~8B/cycle,
            3-operand ops run 1 elem/cycle in bf16 but ~0.4 in f32)."""
            u4 = upool.tile([P, 4, W], BF16, name="u")
            x4 = xs[:, r0 + 1:r0 + 5, :]
            y4 = ys[:, r0:r0 + 4, :]
            v4 = vpool.tile([P, 4, W], BF16, name="v")
            m4 = mpool.tile([P, 4, W], BF16, name="m")
            nc.scalar.activation(
                v4, ps4, AF.Identity,
                bias=pt[:, PB0:PB0 + 1], scale=pt[:, PS:PS + 1],
            )
            nc.vector.tensor_scalar(
                m4, v4, 0.0, pt[:, PCM:PCM + 1], ALU.min, ALU.mult,
            )
            nc.vector.tensor_tensor(u4, v4, m4, ALU.add)
            nc.vector.scalar_tensor_tensor(
                y4, x4, pt[:, PZ:PZ + 1], u4, ALU.add, ALU.add,
            )

        def relu_group(ps4):
            """ACT drain: r = (1-slope) * relu(s*ps + b0) with b0 == 0."""
            r4 = vpool.tile([P, 4, W], BF16, name="r")
            nc.scalar.activation(
                r4, ps4, AF.Relu,
                bias=pt[:, PZ:PZ + 1], scale=pt[:, PSAM:PSAM + 1],
            )
            return r4

        def finish_group(pend):
            """Deferred completion of a 4-row group: fold the residual into
            PSUM (PE diag matmul, even groups) or add it on DVE (odd
            groups), then one stt emits y = slope*s*ps + r."""
            g, s, xs, ys, ps4, r4, r0 = pend
            x4 = xs[:, r0 + 1:r0 + 5, :]
            y4 = ys[:, r0:r0 + 4, :]
            # the final strip always folds x through the PE so its tail is
            # a single stt per group instead of a serial DVE pair
            if g % RES_DEN < RES_NUM or s == NST - 1:
                for half in range(2):
                    nc.tensor.matmul(
                        ps4[:, 2 * half:2 * half + 2, :], lhsT=wx,
                        rhs=x4[:, 2 * half:2 * half + 2, :],
                        start=False, stop=True, skip_group_check=True,
                    )
                nc.vector.scalar_tensor_tensor(
                    y4, ps4, pt[:, PSL:PSL + 1], r4, ALU.mult, ALU.add,
                )
            else:
                t4 = upool.tile([P, 4, W], BF16, name="t")
                nc.vector.scalar_tensor_tensor(
                    t4, ps4, pt[:, PSL:PSL + 1], r4, ALU.mult, ALU.add,
                )
                nc.vector.scalar_tensor_tensor(
                    y4, x4, pt[:, PZ:PZ + 1], t4, ALU.add, ALU.add,
                )
            # store every 8 rows on the Pool HWDGE ring (every 4 in the
            # final strip so the last transfer is issued as early as
            # possible before the teardown barrier)
            h0 = H0S[s]
            r1 = r0 + 4
            if r1 % 8 == 0 or r1 == STRIP_HS[s] or s == NST - 1:
                r = r0 if s == NST - 1 else 8 * ((r1 - 1) // 8)
                nc.gpsimd.dma_start(out=y3[:, h0 + r:h0 + r1, :],
                                    in_=ys[:, r:r1, :])

        # strip 0 signs rows 0 .. STRIP_HS[0]+1 (no top halo); the first
        # chunk covers exactly what conv group 0 consumes
        first_chunks = [6, 8]
        left = STRIP_HS[0] + 1 - sum(first_chunks)
        while left > 0:
            c = min(12, left)
            first_chunks.append(c)
            left -= c
        cur = load_strip(0)
        for c0, sz in zip(np.cumsum([0] + first_chunks[:-1]), first_chunks):
            sign_chunk(0, *cur, int(c0), sz)
        nxt = None
        pend = None
        g = 0                      # global 4-row group index
        for s in range(NST):
            h0 = H0S[s]
            HS_S = STRIP_HS[s]
            NG = HS_S // 4
            xs, act = cur
            ys = ypool.tile([P, HSMAX, W], BF16, name="ys")
            # next-strip sign chunks, interleaved into the ACT queue
            sign_plan = {}
            if s + 1 < NST:
                nchunks = _sign_chunks(STRIP_HS[s + 1] + 2, 6)
                offs = np.cumsum((0,) + nchunks[:-1])
                for j, (c0, sz) in enumerate(zip(offs, nchunks)):
                    emit_k = NG - (len(nchunks) - j) - 3
                    sign_plan.setdefault(max(emit_k, 1), []).append(
                        (int(c0), sz))
            for k in range(NG):
                ps4 = pspool.tile([P, 4, W], F32, name="ps")
                conv_group(act, ps4, 4 * k)
                if k == (0 if s < 2 else 1) and s + 1 < NST:
                    nxt = load_strip(s + 1)   # loads overlap this strip
                # previous group's x-fold matmuls land behind this group's
                # conv in the PE queue, so the PE never waits on its drain
                if pend is not None:
                    finish_group(pend)
                    pend = None
                if FAST_POST:
                    r4 = relu_group(ps4)
                    pend = (g, s, xs, ys, ps4, r4, 4 * k)
                else:
                    post_general(g, xs, ys, ps4, 4 * k)
                    r1 = 4 * k + 4
                    if r1 % 8 == 0 or r1 == HS_S:
                        r = 8 * ((r1 - 1) // 8)
                        nc.gpsimd.dma_start(
                            out=y3[:, h0 + r:h0 + r1, :], in_=ys[:, r:r1, :])
                for c0, sz in sign_plan.pop(k, ()):
                    sign_chunk(s + 1, *nxt, c0, sz)
                g += 1
            for emit_k in sorted(sign_plan):
                for c0, sz in sign_plan[emit_k]:
                    sign_chunk(s + 1, *nxt, c0, sz)
            cur = nxt
        if pend is not None:
            finish_group(pend)


def build_nc():
    nc = bacc.Bacc("TRN2", target_bir_lowering=False, debug=False,
                   num_devices=NCORES)
    x_d = nc.dram_tensor("xin", [P, H * W], BF16, kind="ExternalInput").ap()
    w_d = nc.dram_tensor("wp", [P, 9 * 128], FP8, kind="ExternalInput").ap()
    p_d = nc.dram_tensor("pp", [P, 8], F32, kind="ExternalInput").ap()
    wx_d = nc.dram_tensor("wx", [P, 128], BF16, kind="ExternalInput").ap()
    y_d = nc.dram_tensor("yout", [P, H * W], BF16, kind="ExternalOutput").ap()
    with tile.TileContext(nc) as tc:
        _emit(tc, nc, x_d, w_d, p_d, y_d, wx_d)
    nc.compile()
    return nc


_NC_CACHE = {}


def _get_nc():
    key = (KH2_DR, FAST_POST, RES_NUM, RES_DEN)
    if key not in _NC_CACHE:
        _NC_CACHE[key] = build_nc()
    return _NC_CACHE[key]


def make_inputs(x, rd_k, rd_b, beta, conv_w, pr_bias0, prelu_w, pr_bias1):
    """Host-side prep: per-channel param table, packed sign weights, shards."""
    k = np.asarray(rd_k, np.float32).reshape(C)
    b = np.asarray(rd_b, np.float32).reshape(C)
    s = np.mean(np.abs(np.asarray(conv_w, np.float32)), axis=(1, 2, 3))
    b0 = np.asarray(pr_bias0, np.float32).reshape(C)
    slope = np.asarray(prelu_w, np.float32).reshape(C)
    b1 = np.asarray(pr_bias1, np.float32).reshape(C)
    cm = slope - 1.0
    # pr_bias1 is folded into the residual input x' = x + b1; the sign
    # threshold compensates: sign(k*x + b) == sign(k*x' + (b - k*b1))
    badj = b - k * b1
    cols = np.stack([k, badj, s, b0, cm, slope * s, s * (1.0 - slope),
                     np.zeros(C, np.float32)], axis=1)
    pp = np.concatenate([cols, cols], axis=0).astype(np.float32)  # [128, 8]

    # diag(1/(slope*s)) for the PE residual fold; the stt multiplies PSUM
    # by slope*s afterwards, so x passes through scaled by
    # slope*s*bf16(1/(slope*s)) = 1 + eps, |eps| <~ 4e-3
    safe = np.maximum(np.abs(slope * s), 1e-30)
    dvec = (np.sign(slope * s) / safe).astype(ml_dtypes.bfloat16)
    wxm = np.zeros((P, P), np.float32)
    wxm[np.arange(P), np.arange(P)] = np.tile(
        dvec.astype(np.float32), 2)
    wx = wxm.astype(ml_dtypes.bfloat16)

    sw = np.sign(np.asarray(conv_w, np.float32)).astype(np.float32)

    def blockdiag(kh, kw):
        S = sw[:, :, kh, kw].T  # [ci, co]
        out = np.zeros((P, P), np.float32)
        out[0:C, 0:C] = S
        out[C:P, C:P] = S
        return out

    wp = np.zeros((P, 9, 128), np.float32)
    for kw in range(3):            # [kw, delta(kh 0/1), m] DoubleRow pairs
        for d in range(2):
            wp[:, kw * 2 + d, :] = blockdiag(d, kw)
    for d in range(2):             # kh=2: [delta(kw 0/1), m] DoubleRow
        wp[:, 6 + d, :] = blockdiag(2, d)
    wp[:, 8, :] = blockdiag(2, 2)  # kh=2, kw=2 plain
    wp = np.ascontiguousarray(wp.reshape(P, 9 * 128)).astype(
        mybir.dt.np(FP8))

    xr = np.asarray(x, np.float32) + b1[None, :, None, None]
    xr = xr.astype(ml_dtypes.bfloat16)
    in_maps = []
    for c in range(NCORES):
        xc = np.ascontiguousarray(xr[2 * c:2 * c + 2]).reshape(P, H * W)
        in_maps.append({"xin": xc, "wp": wp, "pp": pp, "wx": wx})
    return in_maps


def kernel(x, rd_k, rd_b, beta, conv_w, pr_bias0, prelu_w, pr_bias1):
    global FAST_POST
    slope = np.asarray(prelu_w, np.float32).reshape(C)
    b0 = np.asarray(pr_bias0, np.float32).reshape(C)
    s = np.mean(np.abs(np.asarray(conv_w, np.float32)), axis=(1, 2, 3))
    if not (np.all(b0 == 0.0) and np.all((slope >= 1e-3) & (slope <= 1.0))
            and np.all(s > 0.0)):
        FAST_POST = False    # relu decomposition needs b0==0, slope in (0,1]
    in_maps = make_inputs(x, rd_k, rd_b, beta, conv_w, pr_bias0, prelu_w,
                          pr_bias1)
    nc = _get_nc()
    res = run_bass_kernel_spmd(nc, in_maps, core_ids=list(range(NCORES)))
    y = np.empty((B, C, H, W), np.float32)
    for c in range(NCORES):
        y[2 * c:2 * c + 2] = np.asarray(
            res.results[c]["yout"]).astype(np.float32).reshape(2, C, H, W)
    return y


# revision 61
# speedup vs baseline: 1.0157x; 1.0157x over previous
"""Trainium2 Bass kernel for nn_BinaryConv2d (B=16, C=64, H=W=256, 3x3, pad 1).

Forward semantics (STE forward values):
  act = sign(x * rd_k + rd_b)                  in {-1, 0, +1}
  bw  = scaling[co] * sign(conv_w)             scaling = mean |conv_w| per out-ch
  y   = conv2d(act, bw, pad=1)
  y   = prelu(y + pr_bias0) + pr_bias1 + x     prelu slope per channel

Strategy: data-parallel over batch, 2 images per core (8 cores).  The two
images' 64 channels are stacked on the 128 SBUF partitions.  x is shipped as
bf16 (residual-precision is ample for the 2e-2 gate) and y is returned as
bf16, halving HBM traffic.  Activations are binarized to fp8 +-1 on the
Scalar engine; the 3x3 conv is 5 PSUM matmuls per output row with
block-diagonal +-1 fp8 weights (exact integer arithmetic in fp32 PSUM):
3 DoubleRow matmuls pair kh=0/1 per kw, a 4th DoubleRow pairs the kh=2
kw=0/1 taps via an overlapping stride-1 rhs AP, and kh=2,kw=2 is a plain
matmul.  Matmuls are emitted weight-major within each 4-row PSUM group so
one LDWEIGHTS covers 4 row-matmuls.  Post-ops per 4-row group: ACT drains
PSUM (v = s*ps + b0, bf16), DVE does the prelu as one stt max(v, slope*v),
and the residual add alternates DVE/Pool.  Measured-cost notes: GPSIMD
cannot read PSUM; Lrelu is broken on TRN2 hw; DVE stt ~1.6ns/elem, Pool
tensor ops ~2.8ns/elem, ACT ~0.9ns/elem incl. overhead.
"""

import sys

if "/opt/trn_rl_repo" not in sys.path:
    sys.path.insert(0, "/opt/trn_rl_repo")

from contextlib import ExitStack

import ml_dtypes
import numpy as np

import concourse.bacc as bacc
import concourse.bass as bass
import concourse.tile as tile
from concourse import mybir
from concourse.ap import AP
from concourse.bass_utils import run_bass_kernel_spmd

B, C, H, W = 16, 64, 256, 256
NCORES = 8
P = 128                      # partitions = 2 images x 64 channels

F32 = mybir.dt.float32
BF16 = mybir.dt.bfloat16
FP8 = mybir.dt.float8e4
AF = mybir.ActivationFunctionType
ALU = mybir.AluOpType
DR = mybir.MatmulPerfMode.DoubleRow

APITCH = 272                 # act row pitch (bytes %16 for DoubleRow AP steps)

# Param table columns (per-partition f32 scalars)
PK, PB, PS, PB0, PCM, PSL, PSAM, PZ = range(8)

# kh=2 row handled as DoubleRow over (kw=0, kw=1) with an overlapping
# stride-1 rhs AP + one plain kw=2 matmul (10W streamed columns per 2-row
# tile).  False falls back to one plain 2-row matmul per kw (12W).
KH2_DR = True
# Fast path (requires pr_bias0 == 0 and 1e-3 <= slope <= 1, true for this
# model's init): prelu(v) = slope*v + (1-slope)*relu(v), so ACT drains
# r = Relu(ps * s*(1-slope)) once, the residual x is folded into PSUM by a
# diag matmul (ps += diag(1/(slope*s)) @ x), and a single DVE stt emits
# y = slope*s*ps + r directly.  kernel() falls back otherwise.
FAST_POST = True
# x-fold engine pattern: groups with g % RES_DEN < RES_NUM put the
# residual through the PE diag matmul, the rest through a DVE stt pair
RES_NUM, RES_DEN = 1, 4

# strip heights (sum == H, all % 4 == 0): small first strip for fast
# pipeline fill, small last strip for a short drain/store tail
STRIP_HS = [32, 36, 36, 36, 36, 36, 36, 8]


def _sign_chunks(nrows, first):
    """Progressive chunk sizes covering nrows of sign activation.  The
    first chunk covers exactly the act rows the strip's first conv group
    consumes, so that group never waits on a later chunk."""
    chunks = [first]
    left = nrows - first
    while left > 0:
        c = min(16, left)
        chunks.append(c)
        left -= c
    return tuple(chunks)


def _emit(tc, nc, x_d, w_d, p_d, y_d, wx_d):
    x3 = x_d.rearrange("p (h w) -> p h w", w=W)
    y3 = y_d.rearrange("p (h w) -> p h w", w=W)

    with ExitStack() as ctx:
        consts = ctx.enter_context(tc.tile_pool(name="consts", bufs=1))
        xpool = ctx.enter_context(tc.tile_pool(name="xpool", bufs=4))
        apool = ctx.enter_context(tc.tile_pool(name="apool", bufs=2))
        ypool = ctx.enter_context(tc.tile_pool(name="ypool", bufs=2))
        vpool = ctx.enter_context(tc.tile_pool(name="vpool", bufs=8))
        mpool = ctx.enter_context(tc.tile_pool(name="mpool", bufs=4))
        upool = ctx.enter_context(tc.tile_pool(name="upool", bufs=8))
        pspool = ctx.enter_context(tc.tile_pool(name="pspool", bufs=4,
                                                space="PSUM"))

        # params first on the load ring (sign needs them); weights on the
        # store ring, which is idle at kernel start
        pt = consts.tile([P, 8], F32)
        nc.sync.dma_start(out=pt, in_=p_d)
        # [kw, delta(kh 0/1), m] DoubleRow weights; kw=0 ships alone so the
        # very first matmul isn't gated on the full weight load
        wdr = consts.tile([P, 3, 2, 128], FP8)
        nc.scalar.dma_start(out=wdr[:, 0], in_=w_d[:, :256].rearrange(
            "p (d m) -> p d m", d=2))
        nc.scalar.dma_start(out=wdr[:, 1:], in_=w_d[:, 256:768].rearrange(
            "p (k d m) -> p k d m", k=2, d=2))
        # kh=2: [delta(kw 0/1), m] DoubleRow + kw=2 plain
        wk2 = consts.tile([P, 2, 128], FP8)
        nc.scalar.dma_start(out=wk2, in_=w_d[:, 768:1024].rearrange(
            "p (d m) -> p d m", d=2))
        wn2 = consts.tile([P, 128], FP8)
        nc.scalar.dma_start(out=wn2, in_=w_d[:, 1024:])
        # diag(1/(slope*s)) for folding the residual into PSUM via the PE
        wx = consts.tile([P, 128], BF16)
        nc.scalar.dma_start(out=wx, in_=wx_d)

        H0S = [sum(STRIP_HS[:i]) for i in range(len(STRIP_HS))]
        NST = len(STRIP_HS)
        HSMAX = max(STRIP_HS)

        def strip_rows(s):
            h0 = H0S[s]
            row_lo = max(h0 - 1, 0)
            row_hi = min(h0 + STRIP_HS[s] + 1, H)
            return h0, row_lo, row_hi, row_lo - (h0 - 1)

        def load_strip(s):
            """DMA the x strip (rows h0-1 .. h0+hs; tile row a <-> global
            h0-1+a) and memset the act padding."""
            h0, row_lo, row_hi, r0 = strip_rows(s)
            nr = row_hi - row_lo
            xs = xpool.tile([P, HSMAX + 2, W], BF16, name="xs")
            if s == 0:
                # first transfer covers exactly the rows conv group 0 needs
                bounds = [0, 6]
                while bounds[-1] < nr:
                    bounds.append(min(bounds[-1] + 9, nr))
            else:
                bounds = [row_lo, row_lo + nr // 2, row_lo + nr]
            for idx, (a, b) in enumerate(zip(bounds, bounds[1:])):
                if b > a:
                    # first startup chunk issues from the Pool ring so it
                    # doesn't queue behind the param-table load
                    eng = nc.gpsimd if (s == 0 and idx == 0) else nc.sync
                    eng.dma_start(out=xs[:, a - (h0 - 1):b - (h0 - 1), :],
                                  in_=x3[:, a:b, :])
            act = apool.tile([P, HSMAX + 2, APITCH], FP8, name="act")
            nrows = STRIP_HS[s] + 2
            nc.gpsimd.memset(act[:, :nrows, 0:1], 0.0)
            nc.gpsimd.memset(act[:, :nrows, W + 1:W + 2], 0.0)
            if s == 0:
                nc.gpsimd.memset(act[:, 0:1, :], 0.0)
            if s == NST - 1:
                nc.gpsimd.memset(act[:, nrows - 1:nrows, :], 0.0)
            return xs, act

        def sign_chunk(s, xs, act, c0, sz):
            """Binarize rows [c0, c0+sz) of the strip's x tile into the
            zero-padded act tile."""
            _, row_lo, row_hi, r0 = strip_rows(s)
            lo = r0 + c0
            hi = min(lo + sz, r0 + (row_hi - row_lo))
            if hi > lo:
                nc.scalar.activation(
                    act[:, lo:hi, 1:W + 1], xs[:, lo:hi, :], AF.Sign,
                    bias=pt[:, PB:PB + 1], scale=pt[:, PK:PK + 1],
                )

        def kh2_overlap_rhs(act, row):
            """[P, 2, W] rhs with both free strides 1: element (d, m) reads
            padded act col d+m of `row`, pairing the kh=2 kw=0/1 taps."""
            base = act[:, row, 0:W]
            return AP(base.tensor, base.offset,
                      [list(base.ap[0]), [1, 2], [1, W]])

        def conv_group(act, ps4, r0):
            """All matmuls for 4 output rows (act tap rows r0..r0+5) in
            weight-major order: consecutive matmuls share a weight set.
            Banks (rows 0-1 / 2-3) start on their first matmul and stop on
            their last.  (Row-pair DR batching via 4-dim rhs APs doesn't
            lower: the 272-element act pitch prevents the free dims from
            flattening to the [P, 2, N] DoubleRow shape.)"""
            for kw in range(3):
                for i in range(4):
                    nc.tensor.matmul(
                        ps4[:, i, :], lhsT=wdr[:, kw, :, :],
                        rhs=act[:, r0 + i:r0 + i + 2, kw:kw + W],
                        start=(kw == 0 and i % 2 == 0),
                        stop=False, perf_mode=DR,
                    )
            if KH2_DR:
                for i in range(4):
                    nc.tensor.matmul(
                        ps4[:, i, :], lhsT=wk2,
                        rhs=kh2_overlap_rhs(act, r0 + i + 2),
                        start=False, stop=False, perf_mode=DR,
                    )
                for half in range(2):
                    nc.tensor.matmul(
                        ps4[:, 2 * half:2 * half + 2, :], lhsT=wn2,
                        rhs=act[:, r0 + 2 * half + 2:r0 + 2 * half + 4,
                                2:2 + W],
                        start=False, stop=True,
                    )
            else:
                for kw in range(3):
                    wsel = wk2[:, 0, :] if kw == 0 else (
                        wk2[:, 1, :] if kw == 1 else wn2)
                    for half in range(2):
                        nc.tensor.matmul(
                            ps4[:, 2 * half:2 * half + 2, :], lhsT=wsel,
                            rhs=act[:, r0 + 2 * half + 2:r0 + 2 * half + 4,
                                    kw:kw + W],
                            start=False, stop=(kw == 2),
                        )

        def post_general(g, xs, ys, ps4, r0):
            """General path: ACT Identity drain, DVE min/mult prelu, DVE
            residual (bf16 intermediates: DVE lane bandwidth is ~8B/cycle,
            3-operand ops run 1 elem/cycle in bf16 but ~0.4 in f32)."""
            u4 = upool.tile([P, 4, W], BF16, name="u")
            x4 = xs[:, r0 + 1:r0 + 5, :]
            y4 = ys[:, r0:r0 + 4, :]
            v4 = vpool.tile([P, 4, W], BF16, name="v")
            m4 = mpool.tile([P, 4, W], BF16, name="m")
            nc.scalar.activation(
                v4, ps4, AF.Identity,
                bias=pt[:, PB0:PB0 + 1], scale=pt[:, PS:PS + 1],
            )
            nc.vector.tensor_scalar(
                m4, v4, 0.0, pt[:, PCM:PCM + 1], ALU.min, ALU.mult,
            )
            nc.vector.tensor_tensor(u4, v4, m4, ALU.add)
            nc.vector.scalar_tensor_tensor(
                y4, x4, pt[:, PZ:PZ + 1], u4, ALU.add, ALU.add,
            )

        def relu_group(ps4):
            """ACT drain: r = (1-slope) * relu(s*ps + b0) with b0 == 0."""
            r4 = vpool.tile([P, 4, W], BF16, name="r")
            nc.scalar.activation(
                r4, ps4, AF.Relu,
                bias=pt[:, PZ:PZ + 1], scale=pt[:, PSAM:PSAM + 1],
            )
            return r4

        def finish_group(pend):
            """Deferred completion of a 4-row group: fold the residual into
            PSUM (PE diag matmul, even groups) or add it on DVE (odd
            groups), then one stt emits y = slope*s*ps + r."""
            g, s, xs, ys, ps4, r4, r0 = pend
            x4 = xs[:, r0 + 1:r0 + 5, :]
            y4 = ys[:, r0:r0 + 4, :]
            # the final strip always folds x through the PE so its tail is
            # a single stt per group instead of a serial DVE pair
            if g % RES_DEN < RES_NUM or s == NST - 1:
                for half in range(2):
                    nc.tensor.matmul(
                        ps4[:, 2 * half:2 * half + 2, :], lhsT=wx,
                        rhs=x4[:, 2 * half:2 * half + 2, :],
                        start=False, stop=True, skip_group_check=True,
                    )
                nc.vector.scalar_tensor_tensor(
                    y4, ps4, pt[:, PSL:PSL + 1], r4, ALU.mult, ALU.add,
                )
            else:
                t4 = upool.tile([P, 4, W], BF16, name="t")
                nc.vector.scalar_tensor_tensor(
                    t4, ps4, pt[:, PSL:PSL + 1], r4, ALU.mult, ALU.add,
                )
                nc.vector.scalar_tensor_tensor(
                    y4, x4, pt[:, PZ:PZ + 1], t4, ALU.add, ALU.add,
                )
            # store every 8 rows on the Pool HWDGE ring (every 4 in the
            # final strip so the last transfer is issued as early as
            # possible before the teardown barrier)
            h0 = H0S[s]
            r1 = r0 + 4
            if r1 % 8 == 0 or r1 == STRIP_HS[s] or s == NST - 1:
                r = r0 if s == NST - 1 else 8 * ((r1 - 1) // 8)
                nc.gpsimd.dma_start(out=y3[:, h0 + r:h0 + r1, :],
                                    in_=ys[:, r:r1, :])

        # strip 0 signs rows 0 .. STRIP_HS[0]+1 (no top halo); the first
        # chunk covers exactly what conv group 0 consumes
        first_chunks = [6, 8]
        left = STRIP_HS[0] + 1 - sum(first_chunks)
        while left > 0:
            c = min(12, left)
            first_chunks.append(c)
            left -= c
        cur = load_strip(0)
        for c0, sz in zip(np.cumsum([0] + first_chunks[:-1]), first_chunks):
            sign_chunk(0, *cur, int(c0), sz)
        nxt = None
        pend = None
        g = 0                      # global 4-row group index
        for s in range(NST):
            h0 = H0S[s]
            HS_S = STRIP_HS[s]
            NG = HS_S // 4
            xs, act = cur
            ys = ypool.tile([P, HSMAX, W], BF16, name="ys")
            # next-strip sign chunks, interleaved into the ACT queue
            sign_plan = {}
            if s + 1 < NST:
                nchunks = _sign_chunks(STRIP_HS[s + 1] + 2, 6)
                offs = np.cumsum((0,) + nchunks[:-1])
                for j, (c0, sz) in enumerate(zip(offs, nchunks)):
                    emit_k = NG - (len(nchunks) - j) - 3
                    sign_plan.setdefault(max(emit_k, 1), []).append(
                        (int(c0), sz))
            for k in range(NG):
                ps4 = pspool.tile([P, 4, W], F32, name="ps")
                conv_group(act, ps4, 4 * k)
                if k == min(1, NG - 1) and s + 1 < NST:
                    nxt = load_strip(s + 1)   # loads overlap this strip
                # previous group's x-fold matmuls land behind this group's
                # conv in the PE queue, so the PE never waits on its drain
                if pend is not None:
                    finish_group(pend)
                    pend = None
                if FAST_POST:
                    r4 = relu_group(ps4)
                    pend = (g, s, xs, ys, ps4, r4, 4 * k)
                else:
                    post_general(g, xs, ys, ps4, 4 * k)
                    r1 = 4 * k + 4
                    if r1 % 8 == 0 or r1 == HS_S:
                        r = 8 * ((r1 - 1) // 8)
                        nc.gpsimd.dma_start(
                            out=y3[:, h0 + r:h0 + r1, :], in_=ys[:, r:r1, :])
                for c0, sz in sign_plan.pop(k, ()):
                    sign_chunk(s + 1, *nxt, c0, sz)
                g += 1
            for emit_k in sorted(sign_plan):
                for c0, sz in sign_plan[emit_k]:
                    sign_chunk(s + 1, *nxt, c0, sz)
            cur = nxt
        if pend is not None:
            finish_group(pend)


def build_nc():
    nc = bacc.Bacc("TRN2", target_bir_lowering=False, debug=False,
                   num_devices=NCORES)
    x_d = nc.dram_tensor("xin", [P, H * W], BF16, kind="ExternalInput").ap()
    w_d = nc.dram_tensor("wp", [P, 9 * 128], FP8, kind="ExternalInput").ap()
    p_d = nc.dram_tensor("pp", [P, 8], F32, kind="ExternalInput").ap()
    wx_d = nc.dram_tensor("wx", [P, 128], BF16, kind="ExternalInput").ap()
    y_d = nc.dram_tensor("yout", [P, H * W], BF16, kind="ExternalOutput").ap()
    with tile.TileContext(nc) as tc:
        _emit(tc, nc, x_d, w_d, p_d, y_d, wx_d)
    nc.compile()
    return nc


_NC_CACHE = {}


def _get_nc():
    key = (KH2_DR, FAST_POST, RES_NUM, RES_DEN)
    if key not in _NC_CACHE:
        _NC_CACHE[key] = build_nc()
    return _NC_CACHE[key]


def make_inputs(x, rd_k, rd_b, beta, conv_w, pr_bias0, prelu_w, pr_bias1):
    """Host-side prep: per-channel param table, packed sign weights, shards."""
    k = np.asarray(rd_k, np.float32).reshape(C)
    b = np.asarray(rd_b, np.float32).reshape(C)
    s = np.mean(np.abs(np.asarray(conv_w, np.float32)), axis=(1, 2, 3))
    b0 = np.asarray(pr_bias0, np.float32).reshape(C)
    slope = np.asarray(prelu_w, np.float32).reshape(C)
    b1 = np.asarray(pr_bias1, np.float32).reshape(C)
    cm = slope - 1.0
    # pr_bias1 is folded into the residual input x' = x + b1; the sign
    # threshold compensates: sign(k*x + b) == sign(k*x' + (b - k*b1))
    badj = b - k * b1
    cols = np.stack([k, badj, s, b0, cm, slope * s, s * (1.0 - slope),
                     np.zeros(C, np.float32)], axis=1)
    pp = np.concatenate([cols, cols], axis=0).astype(np.float32)  # [128, 8]

    # diag(1/(slope*s)) for the PE residual fold; the stt multiplies PSUM
    # by slope*s afterwards, so x passes through scaled by
    # slope*s*bf16(1/(slope*s)) = 1 + eps, |eps| <~ 4e-3
    safe = np.maximum(np.abs(slope * s), 1e-30)
    dvec = (np.sign(slope * s) / safe).astype(ml_dtypes.bfloat16)
    wxm = np.zeros((P, P), np.float32)
    wxm[np.arange(P), np.arange(P)] = np.tile(
        dvec.astype(np.float32), 2)
    wx = wxm.astype(ml_dtypes.bfloat16)

    sw = np.sign(np.asarray(conv_w, np.float32)).astype(np.float32)

    def blockdiag(kh, kw):
        S = sw[:, :, kh, kw].T  # [ci, co]
        out = np.zeros((P, P), np.float32)
        out[0:C, 0:C] = S
        out[C:P, C:P] = S
        return out

    wp = np.zeros((P, 9, 128), np.float32)
    for kw in range(3):            # [kw, delta(kh 0/1), m] DoubleRow pairs
        for d in range(2):
            wp[:, kw * 2 + d, :] = blockdiag(d, kw)
    for d in range(2):             # kh=2: [delta(kw 0/1), m] DoubleRow
        wp[:, 6 + d, :] = blockdiag(2, d)
    wp[:, 8, :] = blockdiag(2, 2)  # kh=2, kw=2 plain
    wp = np.ascontiguousarray(wp.reshape(P, 9 * 128)).astype(
        mybir.dt.np(FP8))

    xr = np.asarray(x, np.float32) + b1[None, :, None, None]
    xr = xr.astype(ml_dtypes.bfloat16)
    in_maps = []
    for c in range(NCORES):
        xc = np.ascontiguousarray(xr[2 * c:2 * c + 2]).reshape(P, H * W)
        in_maps.append({"xin": xc, "wp": wp, "pp": pp, "wx": wx})
    return in_maps


def kernel(x, rd_k, rd_b, beta, conv_w, pr_bias0, prelu_w, pr_bias1):
    global FAST_POST
    slope = np.asarray(prelu_w, np.float32).reshape(C)
    b0 = np.asarray(pr_bias0, np.float32).reshape(C)
    s = np.mean(np.abs(np.asarray(conv_w, np.float32)), axis=(1, 2, 3))
    if not (np.all(b0 == 0.0) and np.all((slope >= 1e-3) & (slope <= 1.0))
            and np.all(s > 0.0)):
        FAST_POST = False    # relu decomposition needs b0==0, slope in (0,1]
    in_maps = make_inputs(x, rd_k, rd_b, beta, conv_w, pr_bias0, prelu_w,
                          pr_bias1)
    nc = _get_nc()
    res = run_bass_kernel_spmd(nc, in_maps, core_ids=list(range(NCORES)))
    y = np.empty((B, C, H, W), np.float32)
    for c in range(NCORES):
        y[2 * c:2 * c + 2] = np.asarray(
            res.results[c]["yout"]).astype(np.float32).reshape(2, C, H, W)
    return y


# revision 70
# speedup vs baseline: 1.0217x; 1.0059x over previous
"""Trainium2 Bass kernel for nn_BinaryConv2d (B=16, C=64, H=W=256, 3x3, pad 1).

Forward semantics (STE forward values):
  act = sign(x * rd_k + rd_b)                  in {-1, 0, +1}
  bw  = scaling[co] * sign(conv_w)             scaling = mean |conv_w| per out-ch
  y   = conv2d(act, bw, pad=1)
  y   = prelu(y + pr_bias0) + pr_bias1 + x     prelu slope per channel

Strategy: data-parallel over batch, 2 images per core (8 cores).  The two
images' 64 channels are stacked on the 128 SBUF partitions.  x is shipped as
bf16 (residual-precision is ample for the 2e-2 gate) and y is returned as
bf16, halving HBM traffic.  Activations are binarized to fp8 +-1 on the
Scalar engine; the 3x3 conv is 5 PSUM matmuls per output row with
block-diagonal +-1 fp8 weights (exact integer arithmetic in fp32 PSUM):
3 DoubleRow matmuls pair kh=0/1 per kw, a 4th DoubleRow pairs the kh=2
kw=0/1 taps via an overlapping stride-1 rhs AP, and kh=2,kw=2 is a plain
matmul.  Matmuls are emitted weight-major within each 4-row PSUM group so
one LDWEIGHTS covers 4 row-matmuls.  Post-ops per 4-row group: ACT drains
PSUM (v = s*ps + b0, bf16), DVE does the prelu as one stt max(v, slope*v),
and the residual add alternates DVE/Pool.  Measured-cost notes: GPSIMD
cannot read PSUM; Lrelu is broken on TRN2 hw; DVE stt ~1.6ns/elem, Pool
tensor ops ~2.8ns/elem, ACT ~0.9ns/elem incl. overhead.
"""

import sys

if "/opt/trn_rl_repo" not in sys.path:
    sys.path.insert(0, "/opt/trn_rl_repo")

from contextlib import ExitStack

import ml_dtypes
import numpy as np

import concourse.bacc as bacc
import concourse.bass as bass
import concourse.tile as tile
from concourse import mybir
from concourse.ap import AP
from concourse.bass_utils import run_bass_kernel_spmd

B, C, H, W = 16, 64, 256, 256
NCORES = 8
P = 128                      # partitions = 2 images x 64 channels

F32 = mybir.dt.float32
BF16 = mybir.dt.bfloat16
FP8 = mybir.dt.float8e4
AF = mybir.ActivationFunctionType
ALU = mybir.AluOpType
DR = mybir.MatmulPerfMode.DoubleRow

APITCH = 272                 # act row pitch (bytes %16 for DoubleRow AP steps)

# Param table columns (per-partition f32 scalars)
PK, PB, PS, PB0, PCM, PSL, PSAM, PZ = range(8)

# kh=2 row handled as DoubleRow over (kw=0, kw=1) with an overlapping
# stride-1 rhs AP + one plain kw=2 matmul (10W streamed columns per 2-row
# tile).  False falls back to one plain 2-row matmul per kw (12W).
KH2_DR = True
# Fast path (requires pr_bias0 == 0 and 1e-3 <= slope <= 1, true for this
# model's init): prelu(v) = slope*v + (1-slope)*relu(v), so ACT drains
# r = Relu(ps * s*(1-slope)) once, the residual x is folded into PSUM by a
# diag matmul (ps += diag(1/(slope*s)) @ x), and a single DVE stt emits
# y = slope*s*ps + r directly.  kernel() falls back otherwise.
FAST_POST = True
# x-fold engine pattern: groups with g % RES_DEN < RES_NUM put the
# residual through the PE diag matmul, the rest through a DVE stt pair
RES_NUM, RES_DEN = 1, 4
# when rd_k and the adjusted sign bias are channel-uniform (true here),
# sign uses float immediates so the first chunk never waits on the
# param-table DMA; set by kernel() before building
SIGN_IMM = None

# strip heights (sum == H, all % 4 == 0): small first strip for fast
# pipeline fill, small last strip for a short drain/store tail
STRIP_HS = [32, 36, 36, 36, 36, 36, 36, 8]


def _sign_chunks(nrows, first):
    """Progressive chunk sizes covering nrows of sign activation.  The
    first chunk covers exactly the act rows the strip's first conv group
    consumes, so that group never waits on a later chunk."""
    chunks = [first]
    left = nrows - first
    while left > 0:
        c = min(16, left)
        chunks.append(c)
        left -= c
    return tuple(chunks)


def _emit(tc, nc, x_d, w_d, p_d, y_d, wx_d):
    x3 = x_d.rearrange("p (h w) -> p h w", w=W)
    y3 = y_d.rearrange("p (h w) -> p h w", w=W)

    with ExitStack() as ctx:
        consts = ctx.enter_context(tc.tile_pool(name="consts", bufs=1))
        xpool = ctx.enter_context(tc.tile_pool(name="xpool", bufs=4))
        apool = ctx.enter_context(tc.tile_pool(name="apool", bufs=2))
        ypool = ctx.enter_context(tc.tile_pool(name="ypool", bufs=2))
        vpool = ctx.enter_context(tc.tile_pool(name="vpool", bufs=8))
        mpool = ctx.enter_context(tc.tile_pool(name="mpool", bufs=4))
        upool = ctx.enter_context(tc.tile_pool(name="upool", bufs=8))
        pspool = ctx.enter_context(tc.tile_pool(name="pspool", bufs=4,
                                                space="PSUM"))

        # params first on the load ring (sign needs them); weights on the
        # store ring, which is idle at kernel start
        pt = consts.tile([P, 8], F32)
        nc.sync.dma_start(out=pt, in_=p_d)
        # [kw, delta(kh 0/1), m] DoubleRow weights; kw=0 ships alone so the
        # very first matmul isn't gated on the full weight load
        wdr = consts.tile([P, 3, 2, 128], FP8)
        nc.scalar.dma_start(out=wdr[:, 0], in_=w_d[:, :256].rearrange(
            "p (d m) -> p d m", d=2))
        nc.scalar.dma_start(out=wdr[:, 1:], in_=w_d[:, 256:768].rearrange(
            "p (k d m) -> p k d m", k=2, d=2))
        # kh=2: [delta(kw 0/1), m] DoubleRow + kw=2 plain
        wk2 = consts.tile([P, 2, 128], FP8)
        nc.scalar.dma_start(out=wk2, in_=w_d[:, 768:1024].rearrange(
            "p (d m) -> p d m", d=2))
        wn2 = consts.tile([P, 128], FP8)
        nc.scalar.dma_start(out=wn2, in_=w_d[:, 1024:])
        # diag(1/(slope*s)) for folding the residual into PSUM via the PE
        wx = consts.tile([P, 128], BF16)
        nc.scalar.dma_start(out=wx, in_=wx_d)

        H0S = [sum(STRIP_HS[:i]) for i in range(len(STRIP_HS))]
        NST = len(STRIP_HS)
        HSMAX = max(STRIP_HS)

        def strip_rows(s):
            h0 = H0S[s]
            row_lo = max(h0 - 1, 0)
            row_hi = min(h0 + STRIP_HS[s] + 1, H)
            return h0, row_lo, row_hi, row_lo - (h0 - 1)

        def load_strip(s):
            """DMA the x strip (rows h0-1 .. h0+hs; tile row a <-> global
            h0-1+a) and memset the act padding."""
            h0, row_lo, row_hi, r0 = strip_rows(s)
            nr = row_hi - row_lo
            xs = xpool.tile([P, HSMAX + 2, W], BF16, name="xs")
            if s == 0:
                # first transfer covers exactly what sign chunk 1 needs
                # (conv group 0's bank 0 only reads act rows 0..3)
                bounds = [0, 4]
                while bounds[-1] < nr:
                    bounds.append(min(bounds[-1] + 10, nr))
            else:
                bounds = [row_lo, row_lo + nr // 2, row_lo + nr]
            for idx, (a, b) in enumerate(zip(bounds, bounds[1:])):
                if b > a:
                    # first startup chunk issues from the Pool ring so it
                    # doesn't queue behind the param-table load
                    eng = nc.gpsimd if (s == 0 and idx == 0) else nc.sync
                    eng.dma_start(out=xs[:, a - (h0 - 1):b - (h0 - 1), :],
                                  in_=x3[:, a:b, :])
            act = apool.tile([P, HSMAX + 2, APITCH], FP8, name="act")
            nrows = STRIP_HS[s] + 2
            nc.gpsimd.memset(act[:, :nrows, 0:1], 0.0)
            nc.gpsimd.memset(act[:, :nrows, W + 1:W + 2], 0.0)
            if s == 0:
                nc.gpsimd.memset(act[:, 0:1, :], 0.0)
            if s == NST - 1:
                nc.gpsimd.memset(act[:, nrows - 1:nrows, :], 0.0)
            return xs, act

        def sign_chunk(s, xs, act, c0, sz):
            """Binarize rows [c0, c0+sz) of the strip's x tile into the
            zero-padded act tile."""
            _, row_lo, row_hi, r0 = strip_rows(s)
            lo = r0 + c0
            hi = min(lo + sz, r0 + (row_hi - row_lo))
            if hi > lo:
                if SIGN_IMM is not None:
                    nc.scalar.activation(
                        act[:, lo:hi, 1:W + 1], xs[:, lo:hi, :], AF.Sign,
                        bias=SIGN_IMM[1], scale=SIGN_IMM[0],
                    )
                else:
                    nc.scalar.activation(
                        act[:, lo:hi, 1:W + 1], xs[:, lo:hi, :], AF.Sign,
                        bias=pt[:, PB:PB + 1], scale=pt[:, PK:PK + 1],
                    )

        def kh2_overlap_rhs(act, row):
            """[P, 2, W] rhs with both free strides 1: element (d, m) reads
            padded act col d+m of `row`, pairing the kh=2 kw=0/1 taps."""
            base = act[:, row, 0:W]
            return AP(base.tensor, base.offset,
                      [list(base.ap[0]), [1, 2], [1, W]])

        def conv_group(act, ps4, r0):
            """All matmuls for 4 output rows (act tap rows r0..r0+5) in
            weight-major order: consecutive matmuls share a weight set.
            Banks (rows 0-1 / 2-3) start on their first matmul and stop on
            their last.  (Row-pair DR batching via 4-dim rhs APs doesn't
            lower: the 272-element act pitch prevents the free dims from
            flattening to the [P, 2, N] DoubleRow shape.)"""
            for kw in range(3):
                for i in range(4):
                    nc.tensor.matmul(
                        ps4[:, i, :], lhsT=wdr[:, kw, :, :],
                        rhs=act[:, r0 + i:r0 + i + 2, kw:kw + W],
                        start=(kw == 0 and i % 2 == 0),
                        stop=False, perf_mode=DR,
                    )
            if KH2_DR:
                for i in range(4):
                    nc.tensor.matmul(
                        ps4[:, i, :], lhsT=wk2,
                        rhs=kh2_overlap_rhs(act, r0 + i + 2),
                        start=False, stop=False, perf_mode=DR,
                    )
                for half in range(2):
                    nc.tensor.matmul(
                        ps4[:, 2 * half:2 * half + 2, :], lhsT=wn2,
                        rhs=act[:, r0 + 2 * half + 2:r0 + 2 * half + 4,
                                2:2 + W],
                        start=False, stop=True,
                    )
            else:
                for kw in range(3):
                    wsel = wk2[:, 0, :] if kw == 0 else (
                        wk2[:, 1, :] if kw == 1 else wn2)
                    for half in range(2):
                        nc.tensor.matmul(
                            ps4[:, 2 * half:2 * half + 2, :], lhsT=wsel,
                            rhs=act[:, r0 + 2 * half + 2:r0 + 2 * half + 4,
                                    kw:kw + W],
                            start=False, stop=(kw == 2),
                        )

        def post_general(g, xs, ys, ps4, r0):
            """General path: ACT Identity drain, DVE min/mult prelu, DVE
            residual (bf16 intermediates: DVE lane bandwidth is ~8B/cycle,
            3-operand ops run 1 elem/cycle in bf16 but ~0.4 in f32)."""
            u4 = upool.tile([P, 4, W], BF16, name="u")
            x4 = xs[:, r0 + 1:r0 + 5, :]
            y4 = ys[:, r0:r0 + 4, :]
            v4 = vpool.tile([P, 4, W], BF16, name="v")
            m4 = mpool.tile([P, 4, W], BF16, name="m")
            nc.scalar.activation(
                v4, ps4, AF.Identity,
                bias=pt[:, PB0:PB0 + 1], scale=pt[:, PS:PS + 1],
            )
            nc.vector.tensor_scalar(
                m4, v4, 0.0, pt[:, PCM:PCM + 1], ALU.min, ALU.mult,
            )
            nc.vector.tensor_tensor(u4, v4, m4, ALU.add)
            nc.vector.scalar_tensor_tensor(
                y4, x4, pt[:, PZ:PZ + 1], u4, ALU.add, ALU.add,
            )

        def relu_group(ps4):
            """ACT drain: r = (1-slope) * relu(s*ps + b0) with b0 == 0."""
            r4 = vpool.tile([P, 4, W], BF16, name="r")
            nc.scalar.activation(
                r4, ps4, AF.Relu,
                bias=pt[:, PZ:PZ + 1], scale=pt[:, PSAM:PSAM + 1],
            )
            return r4

        def finish_group(pend):
            """Deferred completion of a 4-row group: fold the residual into
            PSUM (PE diag matmul, even groups) or add it on DVE (odd
            groups), then one stt emits y = slope*s*ps + r."""
            g, s, xs, ys, ps4, r4, r0 = pend
            x4 = xs[:, r0 + 1:r0 + 5, :]
            y4 = ys[:, r0:r0 + 4, :]
            # the final strip always folds x through the PE so its tail is
            # a single stt per group instead of a serial DVE pair
            if g % RES_DEN < RES_NUM or s == NST - 1:
                for half in range(2):
                    nc.tensor.matmul(
                        ps4[:, 2 * half:2 * half + 2, :], lhsT=wx,
                        rhs=x4[:, 2 * half:2 * half + 2, :],
                        start=False, stop=True, skip_group_check=True,
                    )
                nc.vector.scalar_tensor_tensor(
                    y4, ps4, pt[:, PSL:PSL + 1], r4, ALU.mult, ALU.add,
                )
            else:
                t4 = upool.tile([P, 4, W], BF16, name="t")
                nc.vector.scalar_tensor_tensor(
                    t4, ps4, pt[:, PSL:PSL + 1], r4, ALU.mult, ALU.add,
                )
                nc.vector.scalar_tensor_tensor(
                    y4, x4, pt[:, PZ:PZ + 1], t4, ALU.add, ALU.add,
                )
            # store every 8 rows on the Pool HWDGE ring (every 4 in the
            # final strip so the last transfer is issued as early as
            # possible before the teardown barrier)
            h0 = H0S[s]
            r1 = r0 + 4
            if r1 % 8 == 0 or r1 == STRIP_HS[s] or s == NST - 1:
                r = r0 if s == NST - 1 else 8 * ((r1 - 1) // 8)
                nc.gpsimd.dma_start(out=y3[:, h0 + r:h0 + r1, :],
                                    in_=ys[:, r:r1, :])

        # strip 0 signs rows 0 .. STRIP_HS[0]+1 (no top halo); the first
        # chunk covers exactly what conv group 0's first PSUM bank consumes
        first_chunks = [4, 8]
        left = STRIP_HS[0] + 1 - sum(first_chunks)
        while left > 0:
            c = min(12, left)
            first_chunks.append(c)
            left -= c
        cur = load_strip(0)
        for c0, sz in zip(np.cumsum([0] + first_chunks[:-1]), first_chunks):
            sign_chunk(0, *cur, int(c0), sz)
        nxt = None
        pend = None
        g = 0                      # global 4-row group index
        for s in range(NST):
            h0 = H0S[s]
            HS_S = STRIP_HS[s]
            NG = HS_S // 4
            xs, act = cur
            ys = ypool.tile([P, HSMAX, W], BF16, name="ys")
            # next-strip sign chunks, interleaved into the ACT queue
            sign_plan = {}
            if s + 1 < NST:
                nchunks = _sign_chunks(STRIP_HS[s + 1] + 2, 6)
                offs = np.cumsum((0,) + nchunks[:-1])
                for j, (c0, sz) in enumerate(zip(offs, nchunks)):
                    # strip 0: one group later, so early sign chunks don't
                    # head-of-line-block the ACT queue while loads land
                    emit_k = NG - (len(nchunks) - j) - (2 if s == 0 else 3)
                    sign_plan.setdefault(max(emit_k, 1), []).append(
                        (int(c0), sz))
            for k in range(NG):
                ps4 = pspool.tile([P, 4, W], F32, name="ps")
                conv_group(act, ps4, 4 * k)
                if k == min(1, NG - 1) and s + 1 < NST:
                    nxt = load_strip(s + 1)   # loads overlap this strip
                # previous group's x-fold matmuls land behind this group's
                # conv in the PE queue, so the PE never waits on its drain
                if pend is not None:
                    finish_group(pend)
                    pend = None
                if FAST_POST:
                    r4 = relu_group(ps4)
                    pend = (g, s, xs, ys, ps4, r4, 4 * k)
                else:
                    post_general(g, xs, ys, ps4, 4 * k)
                    r1 = 4 * k + 4
                    if r1 % 8 == 0 or r1 == HS_S:
                        r = 8 * ((r1 - 1) // 8)
                        nc.gpsimd.dma_start(
                            out=y3[:, h0 + r:h0 + r1, :], in_=ys[:, r:r1, :])
                for c0, sz in sign_plan.pop(k, ()):
                    sign_chunk(s + 1, *nxt, c0, sz)
                g += 1
            for emit_k in sorted(sign_plan):
                for c0, sz in sign_plan[emit_k]:
                    sign_chunk(s + 1, *nxt, c0, sz)
            cur = nxt
        if pend is not None:
            finish_group(pend)


def build_nc():
    nc = bacc.Bacc("TRN2", target_bir_lowering=False, debug=False,
                   num_devices=NCORES)
    x_d = nc.dram_tensor("xin", [P, H * W], BF16, kind="ExternalInput").ap()
    w_d = nc.dram_tensor("wp", [P, 9 * 128], FP8, kind="ExternalInput").ap()
    p_d = nc.dram_tensor("pp", [P, 8], F32, kind="ExternalInput").ap()
    wx_d = nc.dram_tensor("wx", [P, 128], BF16, kind="ExternalInput").ap()
    y_d = nc.dram_tensor("yout", [P, H * W], BF16, kind="ExternalOutput").ap()
    with tile.TileContext(nc) as tc:
        _emit(tc, nc, x_d, w_d, p_d, y_d, wx_d)
    nc.compile()
    return nc


_NC_CACHE = {}


def _get_nc():
    key = (KH2_DR, FAST_POST, RES_NUM, RES_DEN, SIGN_IMM)
    if key not in _NC_CACHE:
        _NC_CACHE[key] = build_nc()
    return _NC_CACHE[key]


def make_inputs(x, rd_k, rd_b, beta, conv_w, pr_bias0, prelu_w, pr_bias1):
    """Host-side prep: per-channel param table, packed sign weights, shards."""
    k = np.asarray(rd_k, np.float32).reshape(C)
    b = np.asarray(rd_b, np.float32).reshape(C)
    s = np.mean(np.abs(np.asarray(conv_w, np.float32)), axis=(1, 2, 3))
    b0 = np.asarray(pr_bias0, np.float32).reshape(C)
    slope = np.asarray(prelu_w, np.float32).reshape(C)
    b1 = np.asarray(pr_bias1, np.float32).reshape(C)
    cm = slope - 1.0
    # pr_bias1 is folded into the residual input x' = x + b1; the sign
    # threshold compensates: sign(k*x + b) == sign(k*x' + (b - k*b1))
    badj = b - k * b1
    global SIGN_IMM
    if np.all(k == k[0]) and np.all(badj == badj[0]):
        SIGN_IMM = (float(k[0]), float(badj[0]))
    else:
        SIGN_IMM = None
    cols = np.stack([k, badj, s, b0, cm, slope * s, s * (1.0 - slope),
                     np.zeros(C, np.float32)], axis=1)
    pp = np.concatenate([cols, cols], axis=0).astype(np.float32)  # [128, 8]

    # diag(1/(slope*s)) for the PE residual fold; the stt multiplies PSUM
    # by slope*s afterwards, so x passes through scaled by
    # slope*s*bf16(1/(slope*s)) = 1 + eps, |eps| <~ 4e-3
    safe = np.maximum(np.abs(slope * s), 1e-30)
    dvec = (np.sign(slope * s) / safe).astype(ml_dtypes.bfloat16)
    wxm = np.zeros((P, P), np.float32)
    wxm[np.arange(P), np.arange(P)] = np.tile(
        dvec.astype(np.float32), 2)
    wx = wxm.astype(ml_dtypes.bfloat16)

    sw = np.sign(np.asarray(conv_w, np.float32)).astype(np.float32)

    def blockdiag(kh, kw):
        S = sw[:, :, kh, kw].T  # [ci, co]
        out = np.zeros((P, P), np.float32)
        out[0:C, 0:C] = S
        out[C:P, C:P] = S
        return out

    wp = np.zeros((P, 9, 128), np.float32)
    for kw in range(3):            # [kw, delta(kh 0/1), m] DoubleRow pairs
        for d in range(2):
            wp[:, kw * 2 + d, :] = blockdiag(d, kw)
    for d in range(2):             # kh=2: [delta(kw 0/1), m] DoubleRow
        wp[:, 6 + d, :] = blockdiag(2, d)
    wp[:, 8, :] = blockdiag(2, 2)  # kh=2, kw=2 plain
    wp = np.ascontiguousarray(wp.reshape(P, 9 * 128)).astype(
        mybir.dt.np(FP8))

    xr = np.asarray(x, np.float32) + b1[None, :, None, None]
    xr = xr.astype(ml_dtypes.bfloat16)
    in_maps = []
    for c in range(NCORES):
        xc = np.ascontiguousarray(xr[2 * c:2 * c + 2]).reshape(P, H * W)
        in_maps.append({"xin": xc, "wp": wp, "pp": pp, "wx": wx})
    return in_maps


def kernel(x, rd_k, rd_b, beta, conv_w, pr_bias0, prelu_w, pr_bias1):
    global FAST_POST
    slope = np.asarray(prelu_w, np.float32).reshape(C)
    b0 = np.asarray(pr_bias0, np.float32).reshape(C)
    s = np.mean(np.abs(np.asarray(conv_w, np.float32)), axis=(1, 2, 3))
    if not (np.all(b0 == 0.0) and np.all((slope >= 1e-3) & (slope <= 1.0))
            and np.all(s > 0.0)):
        FAST_POST = False    # relu decomposition needs b0==0, slope in (0,1]

    in_maps = make_inputs(x, rd_k, rd_b, beta, conv_w, pr_bias0, prelu_w,
                          pr_bias1)
    nc = _get_nc()
    res = run_bass_kernel_spmd(nc, in_maps, core_ids=list(range(NCORES)))
    y = np.empty((B, C, H, W), np.float32)
    for c in range(NCORES):
        y[2 * c:2 * c + 2] = np.asarray(
            res.results[c]["yout"]).astype(np.float32).reshape(2, C, H, W)
    return y


# revision 71
# speedup vs baseline: 1.0466x; 1.0244x over previous
"""Trainium2 Bass kernel for nn_BinaryConv2d (B=16, C=64, H=W=256, 3x3, pad 1).

Forward semantics (STE forward values):
  act = sign(x * rd_k + rd_b)                  in {-1, 0, +1}
  bw  = scaling[co] * sign(conv_w)             scaling = mean |conv_w| per out-ch
  y   = conv2d(act, bw, pad=1)
  y   = prelu(y + pr_bias0) + pr_bias1 + x     prelu slope per channel

Strategy: data-parallel over batch, 2 images per core (8 cores).  The two
images' 64 channels are stacked on the 128 SBUF partitions.  x is shipped as
bf16 (residual-precision is ample for the 2e-2 gate) and y is returned as
bf16, halving HBM traffic.  Activations are binarized to fp8 +-1 on the
Scalar engine; the 3x3 conv is 5 PSUM matmuls per output row with
block-diagonal +-1 fp8 weights (exact integer arithmetic in fp32 PSUM):
3 DoubleRow matmuls pair kh=0/1 per kw, a 4th DoubleRow pairs the kh=2
kw=0/1 taps via an overlapping stride-1 rhs AP, and kh=2,kw=2 is a plain
matmul.  Matmuls are emitted weight-major within each 4-row PSUM group so
one LDWEIGHTS covers 4 row-matmuls.  Post-ops per 4-row group: ACT drains
PSUM (v = s*ps + b0, bf16), DVE does the prelu as one stt max(v, slope*v),
and the residual add alternates DVE/Pool.  Measured-cost notes: GPSIMD
cannot read PSUM; Lrelu is broken on TRN2 hw; DVE stt ~1.6ns/elem, Pool
tensor ops ~2.8ns/elem, ACT ~0.9ns/elem incl. overhead.
"""

import sys

if "/opt/trn_rl_repo" not in sys.path:
    sys.path.insert(0, "/opt/trn_rl_repo")

from contextlib import ExitStack

import ml_dtypes
import numpy as np

import concourse.bacc as bacc
import concourse.bass as bass
import concourse.tile as tile
from concourse import mybir
from concourse.ap import AP
from concourse.bass_utils import run_bass_kernel_spmd

B, C, H, W = 16, 64, 256, 256
NCORES = 8
P = 128                      # partitions = 2 images x 64 channels

F32 = mybir.dt.float32
BF16 = mybir.dt.bfloat16
FP8 = mybir.dt.float8e4
AF = mybir.ActivationFunctionType
ALU = mybir.AluOpType
DR = mybir.MatmulPerfMode.DoubleRow

APITCH = 272                 # act row pitch (bytes %16 for DoubleRow AP steps)

# Param table columns (per-partition f32 scalars)
PK, PB, PS, PB0, PCM, PSL, PSAM, PZ = range(8)

# kh=2 row handled as DoubleRow over (kw=0, kw=1) with an overlapping
# stride-1 rhs AP + one plain kw=2 matmul (10W streamed columns per 2-row
# tile).  False falls back to one plain 2-row matmul per kw (12W).
KH2_DR = True
# Fast path (requires pr_bias0 == 0 and 1e-3 <= slope <= 1, true for this
# model's init): prelu(v) = slope*v + (1-slope)*relu(v), so ACT drains
# r = Relu(ps * s*(1-slope)) once, the residual x is folded into PSUM by a
# diag matmul (ps += diag(1/(slope*s)) @ x), and a single DVE stt emits
# y = slope*s*ps + r directly.  kernel() falls back otherwise.
FAST_POST = True
# x-fold engine pattern: groups with g % RES_DEN < RES_NUM put the
# residual through the PE diag matmul, the rest through a DVE stt pair
RES_NUM, RES_DEN = 1, 4
# when rd_k and the adjusted sign bias are channel-uniform (true here),
# sign uses float immediates so the first chunk never waits on the
# param-table DMA; set by kernel() before building
SIGN_IMM = None

# strip heights (sum == H, all % 4 == 0): small first strip for fast
# pipeline fill, small last strip for a short drain/store tail
STRIP_HS = [32, 36, 36, 36, 36, 36, 36, 8]


def _sign_chunks(nrows, first):
    """Progressive chunk sizes covering nrows of sign activation.  The
    first chunk covers exactly the act rows the strip's first conv group
    consumes, so that group never waits on a later chunk."""
    chunks = [first]
    left = nrows - first
    while left > 0:
        c = min(16, left)
        chunks.append(c)
        left -= c
    return tuple(chunks)


def _emit(tc, nc, x_d, w_d, p_d, y_d, wx_d):
    x3 = x_d.rearrange("p (h w) -> p h w", w=W)
    y3 = y_d.rearrange("p (h w) -> p h w", w=W)

    with ExitStack() as ctx:
        consts = ctx.enter_context(tc.tile_pool(name="consts", bufs=1))
        xpool = ctx.enter_context(tc.tile_pool(name="xpool", bufs=4))
        apool = ctx.enter_context(tc.tile_pool(name="apool", bufs=2))
        ypool = ctx.enter_context(tc.tile_pool(name="ypool", bufs=2))
        vpool = ctx.enter_context(tc.tile_pool(name="vpool", bufs=8))
        mpool = ctx.enter_context(tc.tile_pool(name="mpool", bufs=4))
        upool = ctx.enter_context(tc.tile_pool(name="upool", bufs=8))
        pspool = ctx.enter_context(tc.tile_pool(name="pspool", bufs=4,
                                                space="PSUM"))

        # params first on the load ring (sign needs them); weights on the
        # store ring, which is idle at kernel start
        pt = consts.tile([P, 8], F32)
        nc.sync.dma_start(out=pt, in_=p_d)
        # [kw, delta(kh 0/1), m] DoubleRow weights; kw=0 ships alone so the
        # very first matmul isn't gated on the full weight load
        wdr = consts.tile([P, 3, 2, 128], FP8)
        nc.scalar.dma_start(out=wdr[:, 0], in_=w_d[:, :256].rearrange(
            "p (d m) -> p d m", d=2))
        nc.scalar.dma_start(out=wdr[:, 1:], in_=w_d[:, 256:768].rearrange(
            "p (k d m) -> p k d m", k=2, d=2))
        # kh=2: [delta(kw 0/1), m] DoubleRow + kw=2 plain
        wk2 = consts.tile([P, 2, 128], FP8)
        nc.scalar.dma_start(out=wk2, in_=w_d[:, 768:1024].rearrange(
            "p (d m) -> p d m", d=2))
        wn2 = consts.tile([P, 128], FP8)
        nc.scalar.dma_start(out=wn2, in_=w_d[:, 1024:])
        # diag(1/(slope*s)) for folding the residual into PSUM via the PE
        wx = consts.tile([P, 128], BF16)
        nc.scalar.dma_start(out=wx, in_=wx_d)

        H0S = [sum(STRIP_HS[:i]) for i in range(len(STRIP_HS))]
        NST = len(STRIP_HS)
        HSMAX = max(STRIP_HS)

        def strip_rows(s):
            h0 = H0S[s]
            row_lo = max(h0 - 1, 0)
            row_hi = min(h0 + STRIP_HS[s] + 1, H)
            return h0, row_lo, row_hi, row_lo - (h0 - 1)

        def load_strip(s):
            """DMA the x strip (rows h0-1 .. h0+hs; tile row a <-> global
            h0-1+a) and memset the act padding."""
            h0, row_lo, row_hi, r0 = strip_rows(s)
            nr = row_hi - row_lo
            xs = xpool.tile([P, HSMAX + 2, W], BF16, name="xs")
            if s == 0:
                # first transfer covers exactly what sign chunk 1 needs
                # (conv group 0's bank 0 only reads act rows 0..3)
                bounds = [0, 4]
                while bounds[-1] < nr:
                    bounds.append(min(bounds[-1] + 10, nr))
            else:
                bounds = [row_lo, row_lo + nr // 2, row_lo + nr]
            for idx, (a, b) in enumerate(zip(bounds, bounds[1:])):
                if b > a:
                    # first startup chunk issues from the Pool ring so it
                    # doesn't queue behind the param-table load
                    eng = nc.gpsimd if (s == 0 and idx == 0) else nc.sync
                    eng.dma_start(out=xs[:, a - (h0 - 1):b - (h0 - 1), :],
                                  in_=x3[:, a:b, :])
            act = apool.tile([P, HSMAX + 2, APITCH], FP8, name="act")
            nrows = STRIP_HS[s] + 2
            nc.gpsimd.memset(act[:, :nrows, 0:1], 0.0)
            nc.gpsimd.memset(act[:, :nrows, W + 1:W + 2], 0.0)
            if s == 0:
                nc.gpsimd.memset(act[:, 0:1, :], 0.0)
            if s == NST - 1:
                nc.gpsimd.memset(act[:, nrows - 1:nrows, :], 0.0)
            return xs, act

        def sign_chunk(s, xs, act, c0, sz):
            """Binarize rows [c0, c0+sz) of the strip's x tile into the
            zero-padded act tile."""
            _, row_lo, row_hi, r0 = strip_rows(s)
            lo = r0 + c0
            hi = min(lo + sz, r0 + (row_hi - row_lo))
            if hi > lo:
                if SIGN_IMM is not None:
                    nc.scalar.activation(
                        act[:, lo:hi, 1:W + 1], xs[:, lo:hi, :], AF.Sign,
                        bias=SIGN_IMM[1], scale=SIGN_IMM[0],
                    )
                else:
                    nc.scalar.activation(
                        act[:, lo:hi, 1:W + 1], xs[:, lo:hi, :], AF.Sign,
                        bias=pt[:, PB:PB + 1], scale=pt[:, PK:PK + 1],
                    )

        def kh2_overlap_rhs(act, row):
            """[P, 2, W] rhs with both free strides 1: element (d, m) reads
            padded act col d+m of `row`, pairing the kh=2 kw=0/1 taps."""
            base = act[:, row, 0:W]
            return AP(base.tensor, base.offset,
                      [list(base.ap[0]), [1, 2], [1, W]])

        def conv_group(act, ps4, r0):
            """All matmuls for 4 output rows (act tap rows r0..r0+5) in
            weight-major order: consecutive matmuls share a weight set.
            Banks (rows 0-1 / 2-3) start on their first matmul and stop on
            their last.  (Row-pair DR batching via 4-dim rhs APs doesn't
            lower: the 272-element act pitch prevents the free dims from
            flattening to the [P, 2, N] DoubleRow shape.)"""
            for kw in range(3):
                for i in range(4):
                    nc.tensor.matmul(
                        ps4[:, i, :], lhsT=wdr[:, kw, :, :],
                        rhs=act[:, r0 + i:r0 + i + 2, kw:kw + W],
                        start=(kw == 0 and i % 2 == 0),
                        stop=False, perf_mode=DR,
                    )
            if KH2_DR:
                for i in range(4):
                    nc.tensor.matmul(
                        ps4[:, i, :], lhsT=wk2,
                        rhs=kh2_overlap_rhs(act, r0 + i + 2),
                        start=False, stop=False, perf_mode=DR,
                    )
                for half in range(2):
                    nc.tensor.matmul(
                        ps4[:, 2 * half:2 * half + 2, :], lhsT=wn2,
                        rhs=act[:, r0 + 2 * half + 2:r0 + 2 * half + 4,
                                2:2 + W],
                        start=False, stop=True,
                    )
            else:
                for kw in range(3):
                    wsel = wk2[:, 0, :] if kw == 0 else (
                        wk2[:, 1, :] if kw == 1 else wn2)
                    for half in range(2):
                        nc.tensor.matmul(
                            ps4[:, 2 * half:2 * half + 2, :], lhsT=wsel,
                            rhs=act[:, r0 + 2 * half + 2:r0 + 2 * half + 4,
                                    kw:kw + W],
                            start=False, stop=(kw == 2),
                        )

        def post_general(g, xs, ys, ps4, r0):
            """General path: ACT Identity drain, DVE min/mult prelu, DVE
            residual (bf16 intermediates: DVE lane bandwidth is ~8B/cycle,
            3-operand ops run 1 elem/cycle in bf16 but ~0.4 in f32)."""
            u4 = upool.tile([P, 4, W], BF16, name="u")
            x4 = xs[:, r0 + 1:r0 + 5, :]
            y4 = ys[:, r0:r0 + 4, :]
            v4 = vpool.tile([P, 4, W], BF16, name="v")
            m4 = mpool.tile([P, 4, W], BF16, name="m")
            nc.scalar.activation(
                v4, ps4, AF.Identity,
                bias=pt[:, PB0:PB0 + 1], scale=pt[:, PS:PS + 1],
            )
            nc.vector.tensor_scalar(
                m4, v4, 0.0, pt[:, PCM:PCM + 1], ALU.min, ALU.mult,
            )
            nc.vector.tensor_tensor(u4, v4, m4, ALU.add)
            nc.vector.scalar_tensor_tensor(
                y4, x4, pt[:, PZ:PZ + 1], u4, ALU.add, ALU.add,
            )

        def relu_group(ps4):
            """ACT drain: r = (1-slope) * relu(s*ps + b0) with b0 == 0."""
            r4 = vpool.tile([P, 4, W], BF16, name="r")
            nc.scalar.activation(
                r4, ps4, AF.Relu,
                bias=pt[:, PZ:PZ + 1], scale=pt[:, PSAM:PSAM + 1],
            )
            return r4

        def finish_group(pend):
            """Deferred completion of a 4-row group: fold the residual into
            PSUM (PE diag matmul, even groups) or add it on DVE (odd
            groups), then one stt emits y = slope*s*ps + r."""
            g, s, xs, ys, ps4, r4, r0 = pend
            x4 = xs[:, r0 + 1:r0 + 5, :]
            y4 = ys[:, r0:r0 + 4, :]
            # the final strip folds x through the PE so its tail is a
            # single stt; elsewhere the PE (the critical engine) is spared:
            # the residual add runs on DVE, with every 4th group's add on
            # the otherwise-idle Pool engine to cap DVE load
            if s == NST - 1:
                for half in range(2):
                    nc.tensor.matmul(
                        ps4[:, 2 * half:2 * half + 2, :], lhsT=wx,
                        rhs=x4[:, 2 * half:2 * half + 2, :],
                        start=False, stop=True, skip_group_check=True,
                    )
                nc.vector.scalar_tensor_tensor(
                    y4, ps4, pt[:, PSL:PSL + 1], r4, ALU.mult, ALU.add,
                )
            else:
                t4 = upool.tile([P, 4, W], BF16, name="t")
                nc.vector.scalar_tensor_tensor(
                    t4, ps4, pt[:, PSL:PSL + 1], r4, ALU.mult, ALU.add,
                )
                if g % RES_DEN < RES_NUM:
                    nc.gpsimd.tensor_tensor(y4, t4, x4, ALU.add)
                else:
                    nc.vector.scalar_tensor_tensor(
                        y4, x4, pt[:, PZ:PZ + 1], t4, ALU.add, ALU.add,
                    )
            # store every 8 rows on the Pool HWDGE ring (every 4 in the
            # final strip so the last transfer is issued as early as
            # possible before the teardown barrier)
            h0 = H0S[s]
            r1 = r0 + 4
            if r1 % 8 == 0 or r1 == STRIP_HS[s] or s == NST - 1:
                r = r0 if s == NST - 1 else 8 * ((r1 - 1) // 8)
                nc.gpsimd.dma_start(out=y3[:, h0 + r:h0 + r1, :],
                                    in_=ys[:, r:r1, :])

        # strip 0 signs rows 0 .. STRIP_HS[0]+1 (no top halo); the first
        # chunk covers exactly what conv group 0's first PSUM bank consumes
        first_chunks = [4, 8]
        left = STRIP_HS[0] + 1 - sum(first_chunks)
        while left > 0:
            c = min(12, left)
            first_chunks.append(c)
            left -= c
        cur = load_strip(0)
        for c0, sz in zip(np.cumsum([0] + first_chunks[:-1]), first_chunks):
            sign_chunk(0, *cur, int(c0), sz)
        nxt = None
        pend = None
        g = 0                      # global 4-row group index
        for s in range(NST):
            h0 = H0S[s]
            HS_S = STRIP_HS[s]
            NG = HS_S // 4
            xs, act = cur
            ys = ypool.tile([P, HSMAX, W], BF16, name="ys")
            # next-strip sign chunks, interleaved into the ACT queue
            sign_plan = {}
            if s + 1 < NST:
                nchunks = _sign_chunks(STRIP_HS[s + 1] + 2, 6)
                offs = np.cumsum((0,) + nchunks[:-1])
                for j, (c0, sz) in enumerate(zip(offs, nchunks)):
                    # strip 0: one group later, so early sign chunks don't
                    # head-of-line-block the ACT queue while loads land
                    emit_k = NG - (len(nchunks) - j) - (2 if s == 0 else 3)
                    sign_plan.setdefault(max(emit_k, 1), []).append(
                        (int(c0), sz))
            for k in range(NG):
                ps4 = pspool.tile([P, 4, W], F32, name="ps")
                conv_group(act, ps4, 4 * k)
                if k == min(1, NG - 1) and s + 1 < NST:
                    nxt = load_strip(s + 1)   # loads overlap this strip
                # previous group's x-fold matmuls land behind this group's
                # conv in the PE queue, so the PE never waits on its drain
                if pend is not None:
                    finish_group(pend)
                    pend = None
                if FAST_POST:
                    r4 = relu_group(ps4)
                    pend = (g, s, xs, ys, ps4, r4, 4 * k)
                else:
                    post_general(g, xs, ys, ps4, 4 * k)
                    r1 = 4 * k + 4
                    if r1 % 8 == 0 or r1 == HS_S:
                        r = 8 * ((r1 - 1) // 8)
                        nc.gpsimd.dma_start(
                            out=y3[:, h0 + r:h0 + r1, :], in_=ys[:, r:r1, :])
                for c0, sz in sign_plan.pop(k, ()):
                    sign_chunk(s + 1, *nxt, c0, sz)
                g += 1
            for emit_k in sorted(sign_plan):
                for c0, sz in sign_plan[emit_k]:
                    sign_chunk(s + 1, *nxt, c0, sz)
            cur = nxt
        if pend is not None:
            finish_group(pend)


def build_nc():
    nc = bacc.Bacc("TRN2", target_bir_lowering=False, debug=False,
                   num_devices=NCORES)
    x_d = nc.dram_tensor("xin", [P, H * W], BF16, kind="ExternalInput").ap()
    w_d = nc.dram_tensor("wp", [P, 9 * 128], FP8, kind="ExternalInput").ap()
    p_d = nc.dram_tensor("pp", [P, 8], F32, kind="ExternalInput").ap()
    wx_d = nc.dram_tensor("wx", [P, 128], BF16, kind="ExternalInput").ap()
    y_d = nc.dram_tensor("yout", [P, H * W], BF16, kind="ExternalOutput").ap()
    with tile.TileContext(nc) as tc:
        _emit(tc, nc, x_d, w_d, p_d, y_d, wx_d)
    nc.compile()
    return nc


_NC_CACHE = {}


def _get_nc():
    key = (KH2_DR, FAST_POST, RES_NUM, RES_DEN, SIGN_IMM)
    if key not in _NC_CACHE:
        _NC_CACHE[key] = build_nc()
    return _NC_CACHE[key]


def make_inputs(x, rd_k, rd_b, beta, conv_w, pr_bias0, prelu_w, pr_bias1):
    """Host-side prep: per-channel param table, packed sign weights, shards."""
    k = np.asarray(rd_k, np.float32).reshape(C)
    b = np.asarray(rd_b, np.float32).reshape(C)
    s = np.mean(np.abs(np.asarray(conv_w, np.float32)), axis=(1, 2, 3))
    b0 = np.asarray(pr_bias0, np.float32).reshape(C)
    slope = np.asarray(prelu_w, np.float32).reshape(C)
    b1 = np.asarray(pr_bias1, np.float32).reshape(C)
    cm = slope - 1.0
    # pr_bias1 is folded into the residual input x' = x + b1; the sign
    # threshold compensates: sign(k*x + b) == sign(k*x' + (b - k*b1))
    badj = b - k * b1
    global SIGN_IMM
    if np.all(k == k[0]) and np.all(badj == badj[0]):
        SIGN_IMM = (float(k[0]), float(badj[0]))
    else:
        SIGN_IMM = None
    cols = np.stack([k, badj, s, b0, cm, slope * s, s * (1.0 - slope),
                     np.zeros(C, np.float32)], axis=1)
    pp = np.concatenate([cols, cols], axis=0).astype(np.float32)  # [128, 8]

    # diag(1/(slope*s)) for the PE residual fold; the stt multiplies PSUM
    # by slope*s afterwards, so x passes through scaled by
    # slope*s*bf16(1/(slope*s)) = 1 + eps, |eps| <~ 4e-3
    safe = np.maximum(np.abs(slope * s), 1e-30)
    dvec = (np.sign(slope * s) / safe).astype(ml_dtypes.bfloat16)
    wxm = np.zeros((P, P), np.float32)
    wxm[np.arange(P), np.arange(P)] = np.tile(
        dvec.astype(np.float32), 2)
    wx = wxm.astype(ml_dtypes.bfloat16)

    sw = np.sign(np.asarray(conv_w, np.float32)).astype(np.float32)

    def blockdiag(kh, kw):
        S = sw[:, :, kh, kw].T  # [ci, co]
        out = np.zeros((P, P), np.float32)
        out[0:C, 0:C] = S
        out[C:P, C:P] = S
        return out

    wp = np.zeros((P, 9, 128), np.float32)
    for kw in range(3):            # [kw, delta(kh 0/1), m] DoubleRow pairs
        for d in range(2):
            wp[:, kw * 2 + d, :] = blockdiag(d, kw)
    for d in range(2):             # kh=2: [delta(kw 0/1), m] DoubleRow
        wp[:, 6 + d, :] = blockdiag(2, d)
    wp[:, 8, :] = blockdiag(2, 2)  # kh=2, kw=2 plain
    wp = np.ascontiguousarray(wp.reshape(P, 9 * 128)).astype(
        mybir.dt.np(FP8))

    xr = np.asarray(x, np.float32) + b1[None, :, None, None]
    xr = xr.astype(ml_dtypes.bfloat16)
    in_maps = []
    for c in range(NCORES):
        xc = np.ascontiguousarray(xr[2 * c:2 * c + 2]).reshape(P, H * W)
        in_maps.append({"xin": xc, "wp": wp, "pp": pp, "wx": wx})
    return in_maps


def kernel(x, rd_k, rd_b, beta, conv_w, pr_bias0, prelu_w, pr_bias1):
    global FAST_POST
    slope = np.asarray(prelu_w, np.float32).reshape(C)
    b0 = np.asarray(pr_bias0, np.float32).reshape(C)
    s = np.mean(np.abs(np.asarray(conv_w, np.float32)), axis=(1, 2, 3))
    if not (np.all(b0 == 0.0) and np.all((slope >= 1e-3) & (slope <= 1.0))
            and np.all(s > 0.0)):
        FAST_POST = False    # relu decomposition needs b0==0, slope in (0,1]

    in_maps = make_inputs(x, rd_k, rd_b, beta, conv_w, pr_bias0, prelu_w,
                          pr_bias1)
    nc = _get_nc()
    res = run_bass_kernel_spmd(nc, in_maps, core_ids=list(range(NCORES)))
    y = np.empty((B, C, H, W), np.float32)
    for c in range(NCORES):
        y[2 * c:2 * c + 2] = np.asarray(
            res.results[c]["yout"]).astype(np.float32).reshape(2, C, H, W)
    return y
